# revision 6
# baseline (speedup 1.0000x reference)
"""Trainium2 Bass kernel for nn_BurgersSolver_75333726371954.

Burgers' equation explicit solver: interpolate u0 [64,512] to a 513-point
grid, run 5000 sequential periodic-stencil steps on [64,512], snapshot every
50th step at every 2nd spatial point -> [64,257,101].

Strategy (pure data parallel, batch sharded 8 rows/core across 8 cores):
  * Affine-scaled state v = C1*u + C2 makes the update constant-free:
        vn[x] = (v[x-1] - v[x+1] + LIN)*v[x] + 2*C2*v[x+1],  LIN = 1-2*C2.
  * A hand-written custom DVE uop (BURGERS_STEP1_ANT) computes one ENTIRE
    time step in ONE single-stream vector instruction: Src0 streams
    v[x+1]; v[x] and v[x-1] are recovered on-chip from a 2-deep ALU-flop
    history chain (CURR_ALU_OUT temporal reads) -- an 8-stage datapath
    program, ~65ns + 1.05ns/elem per instruction.
  * Layout [128 partitions = 8 batch rows x 16 spatial chunks of 32,
    free = scratch + 2H left ghosts + 32 core + H right ghosts + scratch].
    Every step is the same full-width instruction; the first TWO output
    elements are garbage and land exactly on the columns the ghost taper
    abandons (left taper 2/step, right taper 1/step). Long constant
    streams keep each instruction's reads clear of the previous one's
    SBUF writeback.
  * Ghost refresh every H steps with two STREAM_SHUFFLE copies (32-lane
    partition permutation, same pattern in all four quadrants) -- no PE,
    no cross-engine semaphores. A wide spacer copy provides writeback
    margin for the shuffle that reads the step's late-written columns.
  * Snapshots: strided copy of the 16 even core columns into an SBUF
    accumulator; one DMA out at the end; host decodes u = (v - C2)/C1.
"""

import numpy as np

# ---- problem constants (hardcoded; must match the reference config) ----
MX = 513
MT = 5001
DX = 1.0 / (MX - 1)
DT = 1.0 / (MT - 1)
C1 = DT / (2.0 * DX)            # 0.0512
C2 = 0.005 * DT / DX ** 2       # 0.262144
LIN = float(1.0 - 2.0 * C2)
TWO_C2 = float(2.0 * C2)

NSTEPS = MT - 1                 # 5000
SNAP_EVERY = 50
NSNAP = NSTEPS // SNAP_EVERY + 1  # 101

NCORES = 8
BPC = 8                         # batch rows per core
NCHUNK = 16                     # spatial chunks per batch row
CH = 32                         # chunk width (NCHUNK*CH == 512)
H = 10                          # steps between ghost exchanges
T = CH + 3 * H + 2              # tile width: scratch + 2H + CH + H + scratch
CORE0 = 1 + 2 * H               # first core column in the tile

_COMPILED = {}

# ---------------------------------------------------------------------------
# custom DVE op: one Burgers step per instruction (single source stream)
# ---------------------------------------------------------------------------

_DVE_OP = {}


def _register_dve_op():
    if "op" in _DVE_OP:
        return _DVE_OP["op"]
    import concourse.dve_ops as dve_ops
    from concourse.dve_spec import Spec, Src0
    from concourse.dve_uop import (
        ENABLE,
        AluInp,
        AluOp,
        DelayInp,
        DveOpSpec,
        InpSel,
        OutPath,
        OutSel,
        Trigger,
        UopConfig,
        UopDpConfig,
    )

    name = "BURGERS_STEP1_ANT"

    def build_uop():
        u = UopConfig()
        u.enable_input(InpSel.SRC_0, 1)    # PREV_DELAY_0 = r = v[x+1]
        u.enable_input(InpSel.CONST_0, 3)  # PREV_DELAY_2 = LIN
        u.enable_input(InpSel.CONST_1, 4)  # PREV_DELAY_3 = 2*C2
        u.require_inp0 = ENABLE
        u.trigger = (Trigger.SRC_TENSOR_DONE, Trigger.NONE, Trigger.NONE)
        u.enable_output(OutSel.ALU_OUT, OutPath.WR0_LO)
        dp = u.datapath_config
        # blk0: flop0 <- r; lane4 <- prev element's flop0 (= c = v[x])
        dp[0] = (
            UopDpConfig()
            .enable_alu(AluOp.BYPASS, AluInp.PREV_DELAY_0, AluInp.PREV_DELAY_0)
            .pass_through_delay(0, 2, 3)
            .enable_delay_from_src(DelayInp.CURR_ALU_OUT, 4)
        )
        # blk1: flop1 <- c; lane5 <- prev element's flop1 (= l = v[x-1])
        dp[1] = (
            UopDpConfig()
            .enable_alu(AluOp.BYPASS, AluInp.PREV_DELAY_4, AluInp.PREV_DELAY_4)
            .pass_through_delay(0, 2, 3, 4)
            .enable_delay_from_src(DelayInp.CURR_ALU_OUT, 5)
        )
        # blk2: d = l - r
        dp[2] = (
            UopDpConfig()
            .enable_alu(AluOp.SUBTRACT, AluInp.PREV_DELAY_5, AluInp.PREV_DELAY_0)
            .pass_through_delay(0, 2, 3, 4)
        )
        # blk3: dl = d + LIN
        dp[3] = (
            UopDpConfig()
            .enable_alu(AluOp.ADD, AluInp.PREV_ALU_OUT, AluInp.PREV_DELAY_2)
            .pass_through_delay(0, 3, 4)
        )
        # blk4: t = dl * c
        dp[4] = (
            UopDpConfig()
            .enable_alu(AluOp.MULTIPLY, AluInp.PREV_ALU_OUT, AluInp.PREV_DELAY_4)
            .pass_through_delay(0, 3)
        )
        # blk5: q = r * 2C2 ; lane0 <- t
        dp[5] = (
            UopDpConfig()
            .enable_alu(AluOp.MULTIPLY, AluInp.PREV_DELAY_0, AluInp.PREV_DELAY_3)
            .enable_delay_from_src(DelayInp.PREV_ALU_OUT, 0)
        )
        # blk6: vn = q + t
        dp[6] = UopDpConfig().enable_alu(
            AluOp.ADD, AluInp.PREV_ALU_OUT, AluInp.PREV_DELAY_0
        )
        dp[7] = UopDpConfig().pass_through_alu()
        return u

    def reference(in0, in1, c0, c1, c2):
        in0 = np.asarray(in0, np.float32)
        c0 = np.float32(np.asarray(c0).reshape(-1)[0] if np.ndim(c0) else c0)
        c1 = np.float32(np.asarray(c1).reshape(-1)[0] if np.ndim(c1) else c1)
        P = in0.shape[0]
        r = in0.reshape(P, -1)
        out = np.zeros_like(r)
        out[:, 2:] = ((r[:, :-2] - r[:, 2:] + c0) * r[:, 1:-1]
                      + c1 * r[:, 2:])
        return out.reshape(in0.shape)

    class HandDveOp(dve_ops.DveOp):
        def compile(self, ver):
            key = (self.name, ver)
            cached = dve_ops._COMPILE_CACHE.get(key)
            if cached is not None:
                return cached
            result = DveOpSpec(
                name=self.name,
                opcode=dve_ops.get_dve_sub_opcode(self.name),
                uops=[build_uop()],
                rd1_en=False,
            )
            result.validate(ver)
            dve_ops._COMPILE_CACHE[key] = result
            return result

    for op in dve_ops.OPS:
        if op.name == name:
            _DVE_OP["op"] = op
            return op
    op = HandDveOp(
        name, Spec(body=Src0 + Src0, reference=reference), subdim=False,
        uops_sha={},
    )
    dve_ops.OPS.append(op)
    dve_ops._SUB_OPCODE_FOR_NAME[name] = 1 + max(
        dve_ops._SUB_OPCODE_FOR_NAME.values()
    )
    assert dve_ops._SUB_OPCODE_FOR_NAME[name] < 0x20
    dve_ops.CUSTOM_DVE_SPECS[name] = op.spec
    _DVE_OP["op"] = op
    return op


# ---------------------------------------------------------------------------
# shuffle masks: 32-lane permutation, lane l = (row&1)*16 + chunk
# ---------------------------------------------------------------------------

def _masks():
    maskL = [(l & 16) | ((l - 1) & 15) for l in range(32)]  # ghost <- chunk c-1
    maskR = [(l & 16) | ((l + 1) & 15) for l in range(32)]  # ghost <- chunk c+1
    return maskL, maskR


# ---------------------------------------------------------------------------
# kernel build
# ---------------------------------------------------------------------------

def _build():
    import concourse.bass as bass
    import concourse.mybir as mybir

    op = _register_dve_op()
    maskL, maskR = _masks()

    F32 = mybir.dt.float32
    nc = bass.Bass()
    x_in = nc.dram_tensor("x", [128, T], F32, kind="ExternalInput")
    y_out = nc.dram_tensor("y", [128, NSNAP * 16], F32, kind="ExternalOutput")

    assert NSTEPS % H == 0 and SNAP_EVERY % H == 0

    with (
        nc.semaphore("dma_sem") as dma_sem,
        nc.semaphore("v_sem") as v_sem,
        nc.sbuf_tensor("U", [128, T], F32) as U,
        nc.sbuf_tensor("SN", [128, NSNAP * 16], F32) as SN,
        nc.sbuf_tensor("SP", [128, 80], F32) as SP,
    ):
        with nc.Block() as block:
            @block.gpsimd
            def _(g):
                g.memset(SP[:], 0.0)
                g.dma_start(U[:], x_in[:]).then_inc(dma_sem, 16)

            @block.vector
            def _(v):
                v.wait_ge(dma_sem, 16)
                core_even = U[:, CORE0:CORE0 + CH:2]
                v.tensor_copy(SN[:, 0:16], core_even)   # t=0 snapshot
                snap = 1
                for t in range(1, NSTEPS + 1):
                    v._custom_dve(
                        op,
                        out=U[:, 1:T - 1],
                        in0=U[:, 2:T],
                        s0=LIN,
                        s1=TWO_C2,
                    )
                    if t % H == 0 and t < NSTEPS:
                        # wide writeback-margin spacer: shufL reads the step's
                        # late-written core-tail columns.  Order L-then-R so
                        # shufR separates shufL's ghost writes from the next
                        # step's early left-ghost reads; the right ghosts are
                        # only read near the end of the next step's stream.
                        v.tensor_copy(SP[:, 0:36], SP[:, 40:76])
                        v.stream_shuffle(
                            U[:, 1:1 + 2 * H],
                            U[:, 1 + CH:1 + CH + 2 * H], maskL)
                        v.stream_shuffle(
                            U[:, CORE0 + CH:CORE0 + CH + H],
                            U[:, CORE0:CORE0 + H], maskR)
                    if t % SNAP_EVERY == 0:
                        if t % H != 0 or t == NSTEPS:
                            v.tensor_copy(SP[:, 0:36], SP[:, 40:76])
                            v.tensor_copy(SP[:, 40:76], SP[:, 0:36])
                        s = v.tensor_copy(SN[:, snap * 16:snap * 16 + 16],
                                          core_even)
                        snap += 1
                        if t == NSTEPS:
                            s.then_inc(v_sem, 1)

            @block.gpsimd
            def _(g):
                g.wait_ge(v_sem, 1)
                g.dma_start(y_out[:], SN[:]).then_inc(dma_sem, 16)
                g.wait_ge(dma_sem, 32)

    mybir.codegen_inst_isa_subclasses(nc)
    return nc


# ---------------------------------------------------------------------------
# host side
# ---------------------------------------------------------------------------

def _interp_init(u0):
    """Replicate the reference's 1D border-padded linear interp, f32."""
    u0 = np.asarray(u0, dtype=np.float32)
    n_in = u0.shape[1]
    X = np.linspace(0.0, 1.0, MX, dtype=np.float32)
    pts = X * np.float32(2.0) - np.float32(1.0)
    idx = (pts + np.float32(1.0)) * np.float32(0.5) * np.float32(n_in - 1)
    idx = np.clip(idx, 0.0, np.float32(n_in - 1))
    i0 = np.floor(idx).astype(np.int32)
    i0 = np.clip(i0, 0, n_in - 2)
    frac = (idx - i0.astype(np.float32)).astype(np.float32)
    u0f = u0[:, i0] * (np.float32(1.0) - frac) + u0[:, i0 + 1] * frac
    return u0f[:, :-1].astype(np.float32)   # [B, 512]


def _tiles(u0):
    """Per-core [128, T] state tiles of v = C1*u + C2 with ghosts filled."""
    u_init = _interp_init(u0)                       # [64, 512]
    v0 = (np.float32(C1) * u_init + np.float32(C2)).astype(np.float32)
    cc, jj = np.meshgrid(np.arange(NCHUNK), np.arange(T), indexing="ij")
    src = (cc * CH + jj - 2 * H - 1) % 512          # [16, T]
    tiles = []
    for core in range(NCORES):
        rows = v0[core * BPC:(core + 1) * BPC]      # [8, 512]
        tiles.append(rows[:, src].astype(np.float32).reshape(128, T))
    return tiles


def kernel(u0):
    from concourse.bass_utils import run_bass_kernel_spmd

    u0 = np.asarray(u0, dtype=np.float32)
    B = u0.shape[0]
    assert B == NCORES * BPC and u0.shape[1] == 512

    in_maps = [{"x": t} for t in _tiles(u0)]

    if "nc" not in _COMPILED:
        _COMPILED["nc"] = _build()
    nc = _COMPILED["nc"]

    res = run_bass_kernel_spmd(nc, in_maps, core_ids=list(range(NCORES)))

    out = np.empty((B, 257, NSNAP), dtype=np.float32)
    inv_c1 = np.float32(1.0 / C1)
    c2 = np.float32(C2)
    for core in range(NCORES):
        y = res.results[core]["y"]                  # [128, NSNAP*16]
        y = y.reshape(BPC, NCHUNK, NSNAP, 16)       # [b, chunk, t, k]
        u = (y - c2) * inv_c1
        # spatial index nx = chunk*16 + k  (covers 0..255)
        out[core * BPC:(core + 1) * BPC, 0:256, :] = (
            u.transpose(0, 1, 3, 2).reshape(BPC, 256, NSNAP))
    out[:, 256, :] = out[:, 0, :]
    return out
